# revision 7
# baseline (speedup 1.0000x reference)
"""GQA attention (B=2, T=2048, D=4096, 32 Q heads / 8 KV heads, RoPE, causal)
distributed over 8 TRN2 NeuronCores.

Sharding: tensor-parallel over heads. Core r owns KV head r and Q heads
4r..4r+3, computes full-T causal attention for them, then an AllGather of
z^T (pipelined over 8 token groups) lets every core apply a column shard
of wo, so each core emits a disjoint [T, 512] slice of the output.

Compute dtype: bf16 on the TensorEngine with fp32 PSUM accumulation.
"""

import numpy as np
import ml_dtypes

import concourse.bass as bass
import concourse.bacc as bacc
import concourse.tile as tile
import concourse.mybir as mybir
from concourse import bass_utils

F32 = mybir.dt.float32
BF16 = mybir.dt.bfloat16
NPBF16 = ml_dtypes.bfloat16

NCORES = 8
B = 2
T = 2048
D = 4096
HQ = 32
HKV = 8
DH = 128
HL = 4          # q heads per core
NG = 8          # token groups per batch (256 tokens each)
GT = 256        # tokens per group per batch
NDT = 32        # D / 128 contraction tiles
EXP_BIAS = -8.0

# within-quadrant pair-swap mask for stream_shuffle (16 <-> 16)
SWAP_MASK = [(i + 16) % 32 for i in range(32)]
MULT = mybir.AluOpType.mult
ADD = mybir.AluOpType.add


def _build_program():
    nc = bacc.Bacc("TRN2", target_bir_lowering=False, debug=False,
                   num_devices=NCORES)

    xT = nc.dram_tensor("xT", [D, B * T], BF16, kind="ExternalInput")
    wq = nc.dram_tensor("wq", [HL, NDT, 128, 128], BF16, kind="ExternalInput")
    wk = nc.dram_tensor("wk", [NDT, 128, 128], BF16, kind="ExternalInput")
    wv = nc.dram_tensor("wv", [NDT, 128, 128], BF16, kind="ExternalInput")
    wo = nc.dram_tensor("wo", [NDT, 128, 512], BF16, kind="ExternalInput")
    cosg = nc.dram_tensor("cosg", [128, T], F32, kind="ExternalInput")
    sinp = nc.dram_tensor("sinp", [128, T], F32, kind="ExternalInput")
    trineg = nc.dram_tensor("trineg", [128, 128], F32, kind="ExternalInput")
    ones = nc.dram_tensor("ones", [128, 1], BF16, kind="ExternalInput")
    nbias = nc.dram_tensor("nbias", [128, 1], F32, kind="ExternalInput")
    out = nc.dram_tensor("out", [B, T, 512], F32, kind="ExternalOutput")

    groups = [list(range(NCORES))]

    from contextlib import ExitStack
    with tile.TileContext(nc) as tc:
        with ExitStack() as stk:
            pool = lambda *a, **k: stk.enter_context(tc.tile_pool(*a, **k))
            constp = pool(name="const", bufs=1)
            xp = pool(name="xsl", bufs=1)
            wqp = pool(name="wqs", bufs=4)
            wkp = pool(name="wks", bufs=2)
            wvp = pool(name="wvs", bufs=2)
            kcp = pool(name="kcp", bufs=32)
            vcp = pool(name="vcp", bufs=32)
            vtp = pool(name="vtt", bufs=2)
            qtp = pool(name="qtp", bufs=2)
            rpp = pool(name="rptmp", bufs=3)
            expp = pool(name="expp", bufs=6)
            rcpp = pool(name="rcpp", bufs=4)
            rbcp = pool(name="rbcp", bufs=4)
            zccp = pool(name="zccp", bufs=2)
            zgp = pool(name="zgp", bufs=8)
            wop = pool(name="wop", bufs=1)
            osbp = pool(name="osbp", bufs=4)
            accp = pool(name="acc", bufs=2, space="PSUM")
            spsp = pool(name="sps", bufs=2, space="PSUM")
            pvpp = pool(name="pvp", bufs=2, space="PSUM")
            rspp = pool(name="rsp", bufs=2, space="PSUM")
            dramp = pool(name="dram", bufs=1, space="DRAM")
            # ---- constants ----
            COS = constp.tile([128, T], F32, name="COS")
            SINP = constp.tile([128, T], F32, name="SINP")
            TRI = constp.tile([128, 128], F32, name="TRI")
            ONE = constp.tile([128, 1], BF16, name="ONE")
            NB = constp.tile([128, 1], F32, name="NB")
            nc.sync.dma_start(NB[:], nbias[:, :])
            nc.sync.dma_start(COS[:], cosg[:, :])
            nc.sync.dma_start(SINP[:], sinp[:, :])
            nc.sync.dma_start(TRI[:], trineg[:, :])
            nc.sync.dma_start(ONE[:], ones[:, :])

            # resident wo (column shard), [128, 32 mt, 512]
            WO = wop.tile([128, NDT, 512], BF16, name="WO")
            wo_r = wo.ap().rearrange("n p f -> p n f")
            for kk in range(4):
                nc.sync.dma_start(WO[:, bass.ts(kk, 8), :],
                                  wo_r[:, bass.ts(kk, 8), :])

            xT_r = xT.ap().rearrange("(n p) (b t) -> p n b t", p=128, b=B)

            kc = {}
            vc = {}
            cc_in = []
            cc_out = []
            for g in range(NG):
                ci = dramp.tile([HL, 128, B, GT], BF16, name=f"cc_in{g}")
                co = dramp.tile([NCORES, HL, 128, B, GT], BF16,
                                name=f"cc_out{g}", addr_space="Shared")
                cc_in.append(ci)
                cc_out.append(co)

            def rope(ps, cos_sl, sinp_sl, outs):
                """ps: [128, B, GT] psum. outs: list of (out_ap, b, lo, ln);
                writes rope(ps)[:, b, lo:lo+ln] to out_ap."""
                w = rpp.tile([128, B, GT], F32, name="rp_w", tag="rp_w")
                tcs = rpp.tile([128, B, GT], F32, name="rp_c", tag="rp_c")
                sh = rpp.tile([128, B, GT], F32, name="rp_s", tag="rp_s")
                nc.vector.tensor_tensor(w[:], ps[:], sinp_sl, MULT)
                nc.vector.stream_shuffle(sh[:], w[:], SWAP_MASK)
                nc.vector.tensor_tensor(tcs[:], ps[:], cos_sl, MULT)
                for (oap, b, lo, ln) in outs:
                    nc.gpsimd.tensor_tensor(
                        oap, tcs[:, b, lo:lo + ln], sh[:, b, lo:lo + ln], ADD)

            for g in range(NG):
                t0 = g * GT
                # ---- x^T slab for this group's tokens ----
                xsl = xp.tile([128, NDT, B, GT], BF16, name="xsl", tag="xsl")
                for kk in range(4):
                    for b in range(B):
                        nc.sync.dma_start(
                            xsl[:, bass.ts(kk, 8), b, :],
                            xT_r[:, bass.ts(kk, 8), b, t0:t0 + GT])

                cos_sl = COS[:, t0:t0 + GT].unsqueeze(1) \
                    .broadcast_to((128, B, GT))
                sinp_sl = SINP[:, t0:t0 + GT].unsqueeze(1) \
                    .broadcast_to((128, B, GT))

                # ---- K projection + rope -> chunk tiles ----
                k_ps = accp.tile([128, B, GT], F32, name="k_ps", tag="acc")
                for dtb in range(4):
                    wkt = wkp.tile([128, 8, 128], BF16, name="wkt", tag="wkt")
                    nc.sync.dma_start(
                        wkt[:],
                        wk.ap()[bass.ts(dtb, 8)].rearrange("n p f -> p n f"))
                    for j in range(8):
                        dt = dtb * 8 + j
                        nc.tensor.matmul(k_ps[:], wkt[:, j, :],
                                         xsl[:, dt, :, :],
                                         start=(dt == 0), stop=(dt == NDT - 1))
                k_outs = []
                for b in range(B):
                    for half in range(2):
                        ktile = kcp.tile([128, 128], BF16,
                                         name=f"kc_{b}_{2*g+half}", tag="kc")
                        kc[(b, 2 * g + half)] = ktile
                        k_outs.append((ktile[:], b, half * 128, 128))
                rope(k_ps, cos_sl, sinp_sl, k_outs)

                # ---- V^T projection -> transpose -> chunk tiles ----
                v_ps = accp.tile([128, B, GT], F32, name="v_ps", tag="acc")
                for dtb in range(4):
                    wvt = wvp.tile([128, 8, 128], BF16, name="wvt", tag="wvt")
                    nc.sync.dma_start(
                        wvt[:],
                        wv.ap()[bass.ts(dtb, 8)].rearrange("n p f -> p n f"))
                    for j in range(8):
                        dt = dtb * 8 + j
                        nc.tensor.matmul(v_ps[:], wvt[:, j, :],
                                         xsl[:, dt, :, :],
                                         start=(dt == 0), stop=(dt == NDT - 1))
                vT = vtp.tile([128, B, GT], BF16, name="vT", tag="vT")
                nc.vector.tensor_copy(vT[:], v_ps[:])
                for b in range(B):
                    for half in range(2):
                        vtile = vcp.tile([128, 128], BF16,
                                         name=f"vc_{b}_{2*g+half}", tag="vc")
                        vc[(b, 2 * g + half)] = vtile
                        nc.sync.dma_start_transpose(
                            vtile[:], vT[:, b, half * 128:half * 128 + 128])

                # ---- Q projection + rope ----
                qt = qtp.tile([128, HL, B, GT], BF16, name="qt", tag="qt")
                for h in range(HL):
                    q_ps = accp.tile([128, B, GT], F32, name="q_ps", tag="acc")
                    for dtb in range(4):
                        wqt = wqp.tile([128, 8, 128], BF16, name="wqt",
                                       tag="wqt")
                        nc.sync.dma_start(
                            wqt[:],
                            wq.ap()[h, bass.ts(dtb, 8)]
                            .rearrange("n p f -> p n f"))
                        for j in range(8):
                            dt = dtb * 8 + j
                            nc.tensor.matmul(q_ps[:], wqt[:, j, :],
                                             xsl[:, dt, :, :],
                                             start=(dt == 0),
                                             stop=(dt == NDT - 1))
                    rope(q_ps, cos_sl, sinp_sl,
                         [(qt[:, h, b, :], b, 0, GT) for b in range(B)])

                # ---- attention for this group's queries ----
                zcc = zccp.tile([128, HL, B, GT], BF16, name="zcc", tag="zcc")
                for b in range(B):
                    pv = [pvpp.tile([128, 2, GT], F32, name=f"pv{hp}",
                                    tag="pv") for hp in range(2)]
                    rs = [rspp.tile([1, 2, GT], F32, name=f"rs{hp}",
                                    tag="rs") for hp in range(2)]
                    nck = 2 * g + 2
                    for ck in range(nck):
                        kt = kc[(b, ck)]
                        vt = vc[(b, ck)]
                        last = (ck == nck - 1)
                        if not last:
                            spans = [(0, 2, 0, GT), (2, 2, 0, GT)]
                        else:
                            spans = [(0, 4, 128, 128)]
                        for si, (h0, nh, qo, ql) in enumerate(spans):
                            s_ps = spsp.tile([128, nh, ql], F32,
                                             name="s_ps", tag="sps")
                            nc.tensor.matmul(s_ps[:], kt[:],
                                             qt[:, h0:h0 + nh, b, qo:qo + ql])
                            # causal mask on diagonal chunks
                            if ck == 2 * g and not last:
                                nc.vector.tensor_tensor(
                                    s_ps[:, :, 0:128], s_ps[:, :, 0:128],
                                    TRI[:].unsqueeze(1).broadcast_to(
                                        (128, nh, 128)), ADD)
                            if last:
                                nc.vector.tensor_tensor(
                                    s_ps[:], s_ps[:],
                                    TRI[:].unsqueeze(1).broadcast_to(
                                        (128, nh, 128)), ADD)
                            ex = expp.tile([128, nh, ql], BF16,
                                           name="ex", tag="ex")
                            nc.scalar.activation(
                                ex[:], s_ps[:],
                                mybir.ActivationFunctionType.Exp,
                                bias=NB[:])
                            for j in range(nh):
                                h = h0 + j
                                # start=True only on the first matmul into
                                # each PSUM bank (bank-wide has_written clear)
                                nc.tensor.matmul(
                                    pv[h // 2][:, h % 2, qo:qo + ql],
                                    vt[:], ex[:, j, :],
                                    start=(ck == 0 and j == 0), stop=last,
                                    skip_group_check=True)
                            if nh == 2:
                                hp = h0 // 2
                                nc.tensor.matmul(
                                    rs[hp][:, :, qo:qo + ql],
                                    ONE[:], ex[:],
                                    start=(ck == 0), stop=last,
                                    skip_group_check=True)
                            else:
                                for j in range(4):
                                    nc.tensor.matmul(
                                        rs[j // 2][:, j % 2, qo:qo + ql],
                                        ONE[:], ex[:, j, :],
                                        start=False, stop=last,
                                        skip_group_check=True)
                    for hp in range(2):
                        rcp = rcpp.tile([1, 2, GT], F32, name="rcp",
                                        tag="rcp")
                        scr = rcpp.tile([1, 2, GT], F32, name="scr",
                                        tag="scr")
                        nc.vector.reciprocal_approx_accurate(
                            rcp[:], rs[hp][:], scr[:])
                        rbc = rbcp.tile([128, 2, GT], F32, name="rbc",
                                        tag="rbc")
                        nc.gpsimd.partition_broadcast(rbc[:], rcp[:])
                        nc.vector.tensor_tensor(
                            zcc[:, 2 * hp:2 * hp + 2, b, :], pv[hp][:],
                            rbc[:], MULT)

                # ---- AllGather z^T for this group ----
                for h in range(HL):
                    nc.gpsimd.dma_start(cc_in[g][h], zcc[:, h, :, :])
                nc.gpsimd.collective_compute(
                    "AllGather", mybir.AluOpType.bypass,
                    replica_groups=groups,
                    ins=[cc_in[g].opt()],
                    outs=[cc_out[g].opt()],
                )

                # ---- wo column shard for this group's tokens ----
                for half_pass in range(2):
                    o_ps = [pvpp.tile([128, 512], F32, name=f"o_ps{tb}",
                                      tag="pv") for tb in range(2)]
                    for mt in range(NDT):
                        zg = zgp.tile([128, B, GT], BF16, name="zg", tag="zg")
                        nc.sync.dma_start(zg[:], cc_out[g][mt // HL, mt % HL])
                        for tb in range(2):
                            tt = half_pass * 2 + tb
                            bq, hq = tt // 2, tt % 2
                            nc.tensor.matmul(
                                o_ps[tb][:],
                                zg[:, bq, hq * 128:hq * 128 + 128],
                                WO[:, mt, :],
                                start=(mt == 0), stop=(mt == NDT - 1))
                    for tb in range(2):
                        tt = half_pass * 2 + tb
                        bq, hq = tt // 2, tt % 2
                        osb = osbp.tile([128, 512], F32, name="osb",
                                        tag="osb")
                        nc.scalar.copy(osb[:], o_ps[tb][:])
                        nc.sync.dma_start(
                            out.ap()[bq, t0 + hq * 128: t0 + hq * 128 + 128,
                                     :],
                            osb[:])

    nc.compile()
    return nc


_NC_CACHE = {}


def _get_program():
    if "nc" not in _NC_CACHE:
        _NC_CACHE["nc"] = _build_program()
    return _NC_CACHE["nc"]


def _perm128():
    """head-dim permutation: new position p <- original dim."""
    p = np.arange(128)
    quad, r = p // 32, p % 32
    j = quad * 16 + (r % 16)
    return 2 * j + (r >= 16).astype(np.int64)


def _host_prep(x, freqs_cos, freqs_sin, wq, wk, wv, wo):
    perm = _perm128()
    scale = 1.0 / np.sqrt(DH)

    xT = np.ascontiguousarray(
        np.asarray(x, np.float32).reshape(B * T, D).T).astype(NPBF16)

    # wq: [D, HQ*DH] -> [HQ, NDT, 128, 128] with permuted cols + qk scale
    wq4 = (np.asarray(wq, np.float32) * scale).reshape(D, HQ, DH)[:, :, perm]
    wq_t = wq4.reshape(NDT, 128, HQ, DH).transpose(2, 0, 1, 3)
    wk4 = np.asarray(wk, np.float32).reshape(D, HKV, DH)[:, :, perm]
    wk_t = wk4.reshape(NDT, 128, HKV, DH).transpose(2, 0, 1, 3)
    wv_t = np.asarray(wv, np.float32).reshape(NDT, 128, HKV, DH) \
        .transpose(2, 0, 1, 3)
    wo_t = np.asarray(wo, np.float32).reshape(NDT, 128, D)

    # rope tables in permuted layout
    j_of_p = (np.arange(128) // 32) * 16 + (np.arange(128) % 32) % 16
    is_imag = (np.arange(128) % 32) >= 16
    cosT = np.asarray(freqs_cos, np.float32).T[j_of_p, :]   # [128, T]
    sinT = np.asarray(freqs_sin, np.float32).T[j_of_p, :]
    # sin_pre[p] = sign(partner(p)) * sin: + on real slots, - on imag slots
    sinp = np.where(is_imag[:, None], -sinT, sinT).astype(np.float32)
    cosT = np.ascontiguousarray(cosT)
    sinp = np.ascontiguousarray(sinp)

    kk, qq = np.meshgrid(np.arange(128), np.arange(128), indexing="ij")
    trineg = np.where(kk <= qq, 0.0, -1e9).astype(np.float32)
    onearr = np.ones((128, 1), np.float32).astype(NPBF16)

    in_maps = []
    for r in range(NCORES):
        in_maps.append({
            "xT": xT,
            "wq": np.ascontiguousarray(
                wq_t[HL * r:HL * r + HL]).astype(NPBF16),
            "wk": np.ascontiguousarray(wk_t[r]).astype(NPBF16),
            "wv": np.ascontiguousarray(wv_t[r]).astype(NPBF16),
            "wo": np.ascontiguousarray(
                wo_t[:, :, 512 * r:512 * r + 512]).astype(NPBF16),
            "cosg": cosT,
            "sinp": sinp,
            "trineg": trineg,
            "ones": onearr,
            "nbias": np.full((128, 1), EXP_BIAS, np.float32),
        })
    return in_maps


def run(inputs, trace=False, **kw):
    in_maps = _host_prep(**{k: np.asarray(v) for k, v in inputs.items()})
    nc = _get_program()
    res = bass_utils.run_bass_kernel_spmd(
        nc, in_maps, core_ids=list(range(NCORES)), trace=trace, **kw)
    full = np.empty((B, T, D), np.float32)
    for r in range(NCORES):
        full[:, :, 512 * r:512 * r + 512] = res.results[r]["out"]
    return full, res


def kernel(**inputs) -> np.ndarray:
    full, _ = run(inputs, trace=False)
    return full


# revision 8
# speedup vs baseline: 1.0990x; 1.0990x over previous
"""GQA attention (B=2, T=2048, D=4096, 32 Q heads / 8 KV heads, RoPE, causal)
distributed over 8 TRN2 NeuronCores.

Sharding: tensor-parallel over heads. Core r owns KV head r and Q heads
4r..4r+3, computes full-T causal attention for them, then an AllGather of
z^T (pipelined over 8 token groups) lets every core apply a column shard
of wo, so each core emits a disjoint [T, 512] slice of the output.

Compute dtype: bf16 on the TensorEngine with fp32 PSUM accumulation.
"""

import numpy as np
import ml_dtypes

import concourse.bass as bass
import concourse.bacc as bacc
import concourse.tile as tile
import concourse.mybir as mybir
from concourse import bass_utils

F32 = mybir.dt.float32
BF16 = mybir.dt.bfloat16
NPBF16 = ml_dtypes.bfloat16

NCORES = 8
B = 2
T = 2048
D = 4096
HQ = 32
HKV = 8
DH = 128
HL = 4          # q heads per core
NG = 8          # token groups per batch (256 tokens each)
GT = 256        # tokens per group per batch
NDT = 32        # D / 128 contraction tiles
EXP_BIAS = -8.0

# within-quadrant pair-swap mask for stream_shuffle (16 <-> 16)
SWAP_MASK = [(i + 16) % 32 for i in range(32)]
MULT = mybir.AluOpType.mult
ADD = mybir.AluOpType.add


def _build_program():
    nc = bacc.Bacc("TRN2", target_bir_lowering=False, debug=False,
                   num_devices=NCORES)

    xT = nc.dram_tensor("xT", [D, B * T], BF16, kind="ExternalInput")
    wq = nc.dram_tensor("wq", [HL, NDT, 128, 128], BF16, kind="ExternalInput")
    wk = nc.dram_tensor("wk", [NDT, 128, 128], BF16, kind="ExternalInput")
    wv = nc.dram_tensor("wv", [NDT, 128, 128], BF16, kind="ExternalInput")
    wo = nc.dram_tensor("wo", [NDT, 128, 512], BF16, kind="ExternalInput")
    cosg = nc.dram_tensor("cosg", [128, T], F32, kind="ExternalInput")
    sinp = nc.dram_tensor("sinp", [128, T], F32, kind="ExternalInput")
    trineg = nc.dram_tensor("trineg", [128, 128], F32, kind="ExternalInput")
    ones = nc.dram_tensor("ones", [128, 1], BF16, kind="ExternalInput")
    nbias = nc.dram_tensor("nbias", [128, 1], F32, kind="ExternalInput")
    out = nc.dram_tensor("out", [B, T, 512], F32, kind="ExternalOutput")

    groups = [list(range(NCORES))]

    from contextlib import ExitStack
    with tile.TileContext(nc) as tc:
        with ExitStack() as stk:
            pool = lambda *a, **k: stk.enter_context(tc.tile_pool(*a, **k))
            constp = pool(name="const", bufs=1)
            xp = pool(name="xsl", bufs=1)
            wqp = pool(name="wqs", bufs=4)
            wkp = pool(name="wks", bufs=2)
            wvp = pool(name="wvs", bufs=2)
            kcp = pool(name="kcp", bufs=32)
            vcp = pool(name="vcp", bufs=32)
            vtp = pool(name="vtt", bufs=2)
            qtp = pool(name="qtp", bufs=2)
            rpp = pool(name="rptmp", bufs=3)
            expp = pool(name="expp", bufs=6)
            rcpp = pool(name="rcpp", bufs=4)
            rbcp = pool(name="rbcp", bufs=4)
            zccp = pool(name="zccp", bufs=2)
            zgp = pool(name="zgp", bufs=8)
            wop = pool(name="wop", bufs=1)
            osbp = pool(name="osbp", bufs=4)
            accp = pool(name="acc", bufs=2, space="PSUM")
            spsp = pool(name="sps", bufs=2, space="PSUM")
            pvpp = pool(name="pvp", bufs=2, space="PSUM")
            rspp = pool(name="rsp", bufs=2, space="PSUM")
            dramp = pool(name="dram", bufs=1, space="DRAM")
            # ---- constants ----
            COS = constp.tile([128, T], F32, name="COS")
            SINP = constp.tile([128, T], F32, name="SINP")
            TRI = constp.tile([128, 128], F32, name="TRI")
            ONE = constp.tile([128, 1], BF16, name="ONE")
            NB = constp.tile([128, 1], F32, name="NB")
            nc.sync.dma_start(NB[:], nbias[:, :])
            nc.sync.dma_start(COS[:], cosg[:, :])
            nc.sync.dma_start(SINP[:], sinp[:, :])
            nc.sync.dma_start(TRI[:], trineg[:, :])
            nc.sync.dma_start(ONE[:], ones[:, :])

            # resident wo (column shard), [128, 32 mt, 512];
            # loaded lazily (first use is one full group later)
            WO = wop.tile([128, NDT, 512], BF16, name="WO")
            wo_r = wo.ap().rearrange("n p f -> p n f")
            wo_loaded = [False]

            def load_wo():
                if not wo_loaded[0]:
                    wo_loaded[0] = True
                    for kk in range(4):
                        nc.sync.dma_start(WO[:, bass.ts(kk, 8), :],
                                          wo_r[:, bass.ts(kk, 8), :])

            xT_r = xT.ap().rearrange("(n p) (b t) -> p n b t", p=128, b=B)

            kc = {}
            vc = {}
            cc_in = []
            cc_out = []
            for g in range(NG):
                ci = dramp.tile([HL, 128, B, GT], BF16, name=f"cc_in{g}")
                co = dramp.tile([NCORES, HL, 128, B, GT], BF16,
                                name=f"cc_out{g}", addr_space="Shared")
                cc_in.append(ci)
                cc_out.append(co)

            def rope(ps, cos_sl, sinp_sl, outs):
                """ps: [128, B, GT] psum. outs: list of (out_ap, b, lo, ln);
                writes rope(ps)[:, b, lo:lo+ln] to out_ap."""
                w = rpp.tile([128, B, GT], F32, name="rp_w", tag="rp_w")
                tcs = rpp.tile([128, B, GT], F32, name="rp_c", tag="rp_c")
                sh = rpp.tile([128, B, GT], F32, name="rp_s", tag="rp_s")
                nc.vector.tensor_tensor(w[:], ps[:], sinp_sl, MULT)
                nc.vector.stream_shuffle(sh[:], w[:], SWAP_MASK)
                nc.vector.tensor_tensor(tcs[:], ps[:], cos_sl, MULT)
                for (oap, b, lo, ln) in outs:
                    nc.gpsimd.tensor_tensor(
                        oap, tcs[:, b, lo:lo + ln], sh[:, b, lo:lo + ln], ADD)

            for g in range(NG):
                t0 = g * GT
                # ---- x^T slab for this group's tokens ----
                xsl = xp.tile([128, NDT, B, GT], BF16, name="xsl", tag="xsl")
                for kk in range(4):
                    for b in range(B):
                        nc.sync.dma_start(
                            xsl[:, bass.ts(kk, 8), b, :],
                            xT_r[:, bass.ts(kk, 8), b, t0:t0 + GT])

                cos_sl = COS[:, t0:t0 + GT].unsqueeze(1) \
                    .broadcast_to((128, B, GT))
                sinp_sl = SINP[:, t0:t0 + GT].unsqueeze(1) \
                    .broadcast_to((128, B, GT))

                # ---- K projection + rope -> chunk tiles ----
                k_ps = accp.tile([128, B, GT], F32, name="k_ps", tag="acc")
                for dtb in range(4):
                    wkt = wkp.tile([128, 8, 128], BF16, name="wkt", tag="wkt")
                    nc.sync.dma_start(
                        wkt[:],
                        wk.ap()[bass.ts(dtb, 8)].rearrange("n p f -> p n f"))
                    for j in range(8):
                        dt = dtb * 8 + j
                        nc.tensor.matmul(k_ps[:], wkt[:, j, :],
                                         xsl[:, dt, :, :],
                                         start=(dt == 0), stop=(dt == NDT - 1))
                k_outs = []
                for b in range(B):
                    for half in range(2):
                        ktile = kcp.tile([128, 128], BF16,
                                         name=f"kc_{b}_{2*g+half}", tag="kc")
                        kc[(b, 2 * g + half)] = ktile
                        k_outs.append((ktile[:], b, half * 128, 128))
                rope(k_ps, cos_sl, sinp_sl, k_outs)

                # ---- V^T projection -> transpose -> chunk tiles ----
                v_ps = accp.tile([128, B, GT], F32, name="v_ps", tag="acc")
                for dtb in range(4):
                    wvt = wvp.tile([128, 8, 128], BF16, name="wvt", tag="wvt")
                    nc.sync.dma_start(
                        wvt[:],
                        wv.ap()[bass.ts(dtb, 8)].rearrange("n p f -> p n f"))
                    for j in range(8):
                        dt = dtb * 8 + j
                        nc.tensor.matmul(v_ps[:], wvt[:, j, :],
                                         xsl[:, dt, :, :],
                                         start=(dt == 0), stop=(dt == NDT - 1))
                vT = vtp.tile([128, B, GT], BF16, name="vT", tag="vT")
                nc.vector.tensor_copy(vT[:], v_ps[:])
                for b in range(B):
                    for half in range(2):
                        vtile = vcp.tile([128, 128], BF16,
                                         name=f"vc_{b}_{2*g+half}", tag="vc")
                        vc[(b, 2 * g + half)] = vtile
                        nc.sync.dma_start_transpose(
                            vtile[:], vT[:, b, half * 128:half * 128 + 128])

                # ---- Q projection + rope ----
                qt = qtp.tile([128, HL, B, GT], BF16, name="qt", tag="qt")
                for h in range(HL):
                    q_ps = accp.tile([128, B, GT], F32, name="q_ps", tag="acc")
                    for dtb in range(4):
                        wqt = wqp.tile([128, 8, 128], BF16, name="wqt",
                                       tag="wqt")
                        nc.sync.dma_start(
                            wqt[:],
                            wq.ap()[h, bass.ts(dtb, 8)]
                            .rearrange("n p f -> p n f"))
                        for j in range(8):
                            dt = dtb * 8 + j
                            nc.tensor.matmul(q_ps[:], wqt[:, j, :],
                                             xsl[:, dt, :, :],
                                             start=(dt == 0),
                                             stop=(dt == NDT - 1))
                    rope(q_ps, cos_sl, sinp_sl,
                         [(qt[:, h, b, :], b, 0, GT) for b in range(B)])

                # ---- attention for this group's queries ----
                zcc = zccp.tile([128, HL, B, GT], BF16, name="zcc", tag="zcc")
                for b in range(B):
                    pv = [pvpp.tile([128, 2, GT], F32, name=f"pv{hp}",
                                    tag="pv") for hp in range(2)]
                    rs = [rspp.tile([1, 2, GT], F32, name=f"rs{hp}",
                                    tag="rs") for hp in range(2)]
                    nck = 2 * g + 2
                    for ck in range(nck):
                        kt = kc[(b, ck)]
                        vt = vc[(b, ck)]
                        last = (ck == nck - 1)
                        if not last:
                            spans = [(0, 2, 0, GT), (2, 2, 0, GT)]
                        else:
                            spans = [(0, 4, 128, 128)]
                        for si, (h0, nh, qo, ql) in enumerate(spans):
                            s_ps = spsp.tile([128, nh, ql], F32,
                                             name="s_ps", tag="sps")
                            nc.tensor.matmul(s_ps[:], kt[:],
                                             qt[:, h0:h0 + nh, b, qo:qo + ql])
                            # causal mask on diagonal chunks
                            if ck == 2 * g and not last:
                                nc.vector.tensor_tensor(
                                    s_ps[:, :, 0:128], s_ps[:, :, 0:128],
                                    TRI[:].unsqueeze(1).broadcast_to(
                                        (128, nh, 128)), ADD)
                            if last:
                                nc.vector.tensor_tensor(
                                    s_ps[:], s_ps[:],
                                    TRI[:].unsqueeze(1).broadcast_to(
                                        (128, nh, 128)), ADD)
                            ex = expp.tile([128, nh, ql], BF16,
                                           name="ex", tag="ex")
                            nc.scalar.activation(
                                ex[:], s_ps[:],
                                mybir.ActivationFunctionType.Exp,
                                bias=NB[:])
                            for j in range(nh):
                                h = h0 + j
                                # start=True only on the first matmul into
                                # each PSUM bank (bank-wide has_written clear)
                                nc.tensor.matmul(
                                    pv[h // 2][:, h % 2, qo:qo + ql],
                                    vt[:], ex[:, j, :],
                                    start=(ck == 0 and j == 0), stop=last,
                                    skip_group_check=True)
                            if nh == 2:
                                hp = h0 // 2
                                nc.tensor.matmul(
                                    rs[hp][:, :, qo:qo + ql],
                                    ONE[:], ex[:],
                                    start=(ck == 0), stop=last,
                                    skip_group_check=True)
                            else:
                                for j in range(4):
                                    nc.tensor.matmul(
                                        rs[j // 2][:, j % 2, qo:qo + ql],
                                        ONE[:], ex[:, j, :],
                                        start=False, stop=last,
                                        skip_group_check=True)
                    for hp in range(2):
                        rcp = rcpp.tile([1, 2, GT], F32, name="rcp",
                                        tag="rcp")
                        scr = rcpp.tile([1, 2, GT], F32, name="scr",
                                        tag="scr")
                        nc.vector.reciprocal_approx_accurate(
                            rcp[:], rs[hp][:], scr[:])
                        rbc = rbcp.tile([128, 2, GT], F32, name="rbc",
                                        tag="rbc")
                        nc.gpsimd.partition_broadcast(rbc[:], rcp[:])
                        nc.vector.tensor_tensor(
                            zcc[:, 2 * hp:2 * hp + 2, b, :], pv[hp][:],
                            rbc[:], MULT)

                # ---- AllGather z^T for this group ----
                for h in range(HL):
                    nc.gpsimd.dma_start(cc_in[g][h], zcc[:, h, :, :])
                nc.gpsimd.collective_compute(
                    "AllGather", mybir.AluOpType.bypass,
                    replica_groups=groups,
                    ins=[cc_in[g].opt()],
                    outs=[cc_out[g].opt()],
                )
                load_wo()

                # ---- wo for group gw (one group behind: its AllGather
                # completed while this group computed, so the PE stream
                # never stalls on the collective) ----
                for gw in ([g - 1] if g > 0 else []) + \
                        ([g] if g == NG - 1 else []):
                    tw = gw * GT
                    for half_pass in range(2):
                        o_ps = [pvpp.tile([128, 512], F32, name=f"o_ps{tb}",
                                          tag="pv") for tb in range(2)]
                        for mt in range(NDT):
                            zg = zgp.tile([128, B, GT], BF16, name="zg",
                                          tag="zg")
                            nc.sync.dma_start(zg[:],
                                              cc_out[gw][mt // HL, mt % HL])
                            for tb in range(2):
                                tt = half_pass * 2 + tb
                                bq, hq = tt // 2, tt % 2
                                nc.tensor.matmul(
                                    o_ps[tb][:],
                                    zg[:, bq, hq * 128:hq * 128 + 128],
                                    WO[:, mt, :],
                                    start=(mt == 0), stop=(mt == NDT - 1))
                        for tb in range(2):
                            tt = half_pass * 2 + tb
                            bq, hq = tt // 2, tt % 2
                            osb = osbp.tile([128, 512], F32, name="osb",
                                            tag="osb")
                            nc.scalar.copy(osb[:], o_ps[tb][:])
                            nc.sync.dma_start(
                                out.ap()[bq,
                                         tw + hq * 128: tw + hq * 128 + 128,
                                         :],
                                osb[:])

    nc.compile()
    return nc


_NC_CACHE = {}


def _get_program():
    if "nc" not in _NC_CACHE:
        _NC_CACHE["nc"] = _build_program()
    return _NC_CACHE["nc"]


def _perm128():
    """head-dim permutation: new position p <- original dim."""
    p = np.arange(128)
    quad, r = p // 32, p % 32
    j = quad * 16 + (r % 16)
    return 2 * j + (r >= 16).astype(np.int64)


def _host_prep(x, freqs_cos, freqs_sin, wq, wk, wv, wo):
    perm = _perm128()
    scale = 1.0 / np.sqrt(DH)

    xT = np.ascontiguousarray(
        np.asarray(x, np.float32).reshape(B * T, D).T).astype(NPBF16)

    # wq: [D, HQ*DH] -> [HQ, NDT, 128, 128] with permuted cols + qk scale
    wq4 = (np.asarray(wq, np.float32) * scale).reshape(D, HQ, DH)[:, :, perm]
    wq_t = wq4.reshape(NDT, 128, HQ, DH).transpose(2, 0, 1, 3)
    wk4 = np.asarray(wk, np.float32).reshape(D, HKV, DH)[:, :, perm]
    wk_t = wk4.reshape(NDT, 128, HKV, DH).transpose(2, 0, 1, 3)
    wv_t = np.asarray(wv, np.float32).reshape(NDT, 128, HKV, DH) \
        .transpose(2, 0, 1, 3)
    wo_t = np.asarray(wo, np.float32).reshape(NDT, 128, D)

    # rope tables in permuted layout
    j_of_p = (np.arange(128) // 32) * 16 + (np.arange(128) % 32) % 16
    is_imag = (np.arange(128) % 32) >= 16
    cosT = np.asarray(freqs_cos, np.float32).T[j_of_p, :]   # [128, T]
    sinT = np.asarray(freqs_sin, np.float32).T[j_of_p, :]
    # sin_pre[p] = sign(partner(p)) * sin: + on real slots, - on imag slots
    sinp = np.where(is_imag[:, None], -sinT, sinT).astype(np.float32)
    cosT = np.ascontiguousarray(cosT)
    sinp = np.ascontiguousarray(sinp)

    kk, qq = np.meshgrid(np.arange(128), np.arange(128), indexing="ij")
    trineg = np.where(kk <= qq, 0.0, -1e9).astype(np.float32)
    onearr = np.ones((128, 1), np.float32).astype(NPBF16)

    in_maps = []
    for r in range(NCORES):
        in_maps.append({
            "xT": xT,
            "wq": np.ascontiguousarray(
                wq_t[HL * r:HL * r + HL]).astype(NPBF16),
            "wk": np.ascontiguousarray(wk_t[r]).astype(NPBF16),
            "wv": np.ascontiguousarray(wv_t[r]).astype(NPBF16),
            "wo": np.ascontiguousarray(
                wo_t[:, :, 512 * r:512 * r + 512]).astype(NPBF16),
            "cosg": cosT,
            "sinp": sinp,
            "trineg": trineg,
            "ones": onearr,
            "nbias": np.full((128, 1), EXP_BIAS, np.float32),
        })
    return in_maps


def run(inputs, trace=False, **kw):
    in_maps = _host_prep(**{k: np.asarray(v) for k, v in inputs.items()})
    nc = _get_program()
    res = bass_utils.run_bass_kernel_spmd(
        nc, in_maps, core_ids=list(range(NCORES)), trace=trace, **kw)
    full = np.empty((B, T, D), np.float32)
    for r in range(NCORES):
        full[:, :, 512 * r:512 * r + 512] = res.results[r]["out"]
    return full, res


def kernel(**inputs) -> np.ndarray:
    full, _ = run(inputs, trace=False)
    return full
